# revision 14
# baseline (speedup 1.0000x reference)
"""Additive (Bahdanau-style) attention on 8 TRN2 NeuronCores.

reference:
    q = queries @ Wq                      (B,Tq,H)
    k = keys @ Wk                         (B,Tk,H)
    scores[b,i,j] = sum_h wv[h] * tanh(q[b,i,h] + k[b,j,h])
    out = softmax(scores) @ values        (B,Tq,Dv)

The (B,Tq,Tk,H) tanh intermediate (134M tanh evals) is replaced by a
separable Fourier expansion of the shift kernel:

    tanh(s) ~= sum_m c_m sin(w_m s)
    tanh(a+b) ~= sum_m c_m [sin(w_m a)cos(w_m b) + cos(w_m a)sin(w_m b)]

so scores becomes one matmul with contraction dim 2*M*H.

M=3 free frequencies [0.4, 1.3, 2.4] give rel-err ~8e-3 end to end
(gate is 2e-2).  m=0 is evaluated directly by ACT Sin (args stay inside
the spline's accurate +-4 range, cos via the +pi/2 bias); m=1,2 go
through the fixed-point range reduction: y=round(x*w*2^16/2pi) int32 on
DVE, ph=y&0xFFFF, ACT sin(2pi/2^16*ph - pi) = -sin(w x) (signs cancel in
products).

Engine budget: ACT does only Sin and Exp (both table sets pinned during
the DMA wait so no mid-kernel ACT_TABLE_LOAD); DVE does projection
evacuation (fp32->fp16), phases, amplitudes, reciprocal and the final
1/rowsum scaling; GpSimd takes a slice of the amplitude scaling; PE gets
junk-matmul warmup so HAM stays at K=8/8.

Sharding: data-parallel over batch B=8, one batch element per core.
"""

import numpy as np
import ml_dtypes

import concourse.bass as bass
import concourse.tile as tile
from concourse import bacc, mybir
from concourse.bass_utils import run_bass_kernel_spmd
from concourse.masks import make_identity

B, TQ, TK = 8, 256, 256
DQ, DK, DV, H = 512, 512, 512, 256

M = 3
OMEGA = np.array([0.4, 1.3, 2.4])
KBITS = 16
MASK = (1 << KBITS) - 1
TWO_PI = 2.0 * np.pi

F32 = mybir.dt.float32
I32 = mybir.dt.int32
BF16 = mybir.dt.bfloat16
FP16 = mybir.dt.float16
AF = mybir.ActivationFunctionType
ALU = mybir.AluOpType


def _fit_coeffs():
    x = np.linspace(0.0, 10.0, 5001)
    w = np.exp(-x * x / 4.0) + 2e-3
    A = np.sin(np.outer(x, OMEGA))
    sw = np.sqrt(w)[:, None]
    c, *_ = np.linalg.lstsq(A * sw, np.tanh(x) * sw[:, 0], rcond=None)
    return c.astype(np.float64)

COEF = _fit_coeffs()

_CACHE = {}


def _build_graph():
    nc = bacc.Bacc("TRN2", target_bir_lowering=False, debug=False,
                   enable_asserts=False, num_devices=B)

    # fp16, pre-shuffled host-side to the exact SBUF layout
    ins = {}
    for nm in ("qsT", "wq", "ksT", "wk"):
        ins[nm] = nc.dram_tensor(nm, (128, 4, 256), FP16,
                                 kind="ExternalInput").ap()
    ins["vals"] = nc.dram_tensor("vals", (128, 2, DV), BF16,
                                 kind="ExternalInput").ap()
    ins["cwv"] = nc.dram_tensor("cwv", (128, M, 2), F32,
                                kind="ExternalInput").ap()
    out = nc.dram_tensor("out", (TQ, DV), F32, kind="ExternalOutput").ap()

    with tile.TileContext(nc) as tc:
        with tc.tile_pool(name="sb", bufs=1) as sb, \
             tc.tile_pool(name="pp", bufs=1, space="PSUM") as pp, \
             tc.tile_pool(name="pj", bufs=1, space="PSUM") as pj, \
             tc.tile_pool(name="ps_sc", bufs=1, space="PSUM") as ps_sc, \
             tc.tile_pool(name="ps_out", bufs=2, space="PSUM") as ps_out:
            _body(nc, tc, sb, pp, pj, ps_sc, None, ps_out, ins, out)
    nc.compile()
    return nc


def _body(nc, tc, sb, pp, pj, ps_sc, _unused, ps_out, ins, out):
    # ---- input DMA: 2-chunk granularity, q side first, vals last ----
    qsT_sb = sb.tile([128, 4, 256], FP16)       # [d%128, dchunk, qi]
    wq_sb = sb.tile([128, 4, 256], FP16)
    ksT_sb = sb.tile([128, 4, 256], FP16)
    wk_sb = sb.tile([128, 4, 256], FP16)
    vals_bf = sb.tile([128, 2, DV], BF16)
    cwv_sb = sb.tile([128, M, 2], F32)
    # q-side chunks lead on the two fast HW queues; the slow GpSimd SWDGE
    # queue only carries the latest-needed inputs (ksT tail, cwv, vals)
    nc.sync.dma_start(qsT_sb[:, 0:2, :], ins["qsT"][:, 0:2, :])
    nc.scalar.dma_start(wq_sb[:, 0:2, :], ins["wq"][:, 0:2, :])
    nc.gpsimd.dma_start(cwv_sb[:], ins["cwv"])
    nc.sync.dma_start(qsT_sb[:, 2:4, :], ins["qsT"][:, 2:4, :])
    nc.scalar.dma_start(wq_sb[:, 2:4, :], ins["wq"][:, 2:4, :])
    nc.gpsimd.dma_start(ksT_sb[:, 2:4, :], ins["ksT"][:, 2:4, :])
    nc.sync.dma_start(wk_sb[:, 0:2, :], ins["wk"][:, 0:2, :])
    nc.scalar.dma_start(ksT_sb[:, 0:2, :], ins["ksT"][:, 0:2, :])
    nc.sync.dma_start(wk_sb[:, 2:4, :], ins["wk"][:, 2:4, :])
    nc.gpsimd.dma_start(vals_bf[:], ins["vals"])

    # HAM warm-up: junk matmuls run during the DMA wait so the PE
    # clock-gate is already at 2.4 GHz when the projections start
    junk = sb.tile([128, 128], BF16)
    nc.vector.memset(junk[:], 1.0)
    ps_warm = pp.tile([128, 128], F32, name="ps_warm", tag="ps")
    for _ in range(40):
        nc.tensor.matmul(ps_warm[:], junk[:], junk[:], start=True, stop=True)
    negpi = sb.tile([128, 1], F32)
    nc.vector.memset(negpi[:], float(-np.pi))
    halfpi = sb.tile([128, 1], F32)
    nc.vector.memset(halfpi[:], float(np.pi / 2))
    # pin BOTH ACT table sets (trig + exp) before the first real ACT op so
    # the table loads happen during the DMA wait
    warmsin = sb.tile([128, 1], F32)
    nc.scalar.activation(warmsin[:], negpi[:], AF.Sin, bias=0.0, scale=0.1)
    warmexp = sb.tile([128, 1], F32)
    nc.scalar.activation(warmexp[:], negpi[:], AF.Exp, bias=0.0, scale=0.1)

    # ---- per-side fused pipeline: proj -> evac -> sins/phases -> amps ----
    # Strict per-side ordering matters: ACT/DVE are strict-FIFO engines, so
    # any k-gated op issued before the last q op head-of-line blocks the
    # q pipeline while the k DMA is still in flight.
    MF = M - 1                                   # free (range-reduced) freqs
    qT = sb.tile([128, 2 * TQ], FP16)           # [h%128, (j, i)]
    kT = sb.tile([128, 2 * TK], FP16)
    yq = sb.tile([128, MF, 2, 2 * TQ], I32)
    yk = sb.tile([128, MF, 2, 2 * TK], I32)
    phq = sb.tile([128, MF, 2, 2 * TQ], I32)
    phk = sb.tile([128, MF, 2, 2 * TK], I32)
    sq = sb.tile([128, M, 2, 2 * TQ], FP16)      # [h%128, m, quad, (j,i)]
    sqs = sb.tile([128, M, 2, 2 * TQ], FP16)     # amp-scaled q factors
    sk = sb.tile([128, M, 2, 2 * TK], FP16)

    SCALE_SIN = float(TWO_PI / (1 << KBITS))

    def side_pipeline(side, w_sb, x_sb, srcT, y, ph, s, n):
        # projections; the PSUM tile persists so ACT m0 sins read it directly
        ps = pj.tile([128, 2, n], F32, name=f"pj{side}", tag=f"pj{side}")
        for j in range(2):
            for d in range(4):
                nc.tensor.matmul(ps[:, j, :], w_sb[:, d, bass.ts(j, 128)],
                                 x_sb[:, d, :],
                                 start=(d == 0), stop=(d == 3))
        # m=0 direct from PSUM (|w0 x| + pi/2 within ACT Sin's good +-4 range)
        for j in range(2):
            nc.scalar.activation(s[:, 0, 0, bass.ts(j, n)], ps[:, j, :], AF.Sin,
                                 bias=0.0, scale=float(OMEGA[0]))
            nc.scalar.activation(s[:, 0, 1, bass.ts(j, n)], ps[:, j, :], AF.Sin,
                                 bias=halfpi[:], scale=float(OMEGA[0]))
        # fp16 evacuation for the DVE phase passes
        for j in range(2):
            nc.vector.tensor_copy(srcT[:, bass.ts(j, n)], ps[:, j, :])
        # amps interleave into the k-side DVE stream: amp(m) only has to
        # beat the matching k-side sin, and by the time the k stream runs
        # the q-side sins that feed it are long done (no head-of-line risk)
        if side == 1:
            amp(0)
        src = srcT[:]
        for mf in range(MF):
            m = mf + 1
            sc = float(OMEGA[m] * (1 << KBITS) / TWO_PI)
            for quad in range(2):
                nc.vector.tensor_scalar(
                    out=y[:, mf, quad, :], in0=src,
                    scalar1=sc, scalar2=float(quad * (1 << (KBITS - 2))),
                    op0=ALU.mult, op1=ALU.add)
            nc.vector.tensor_scalar(
                out=ph[:, mf, :, :], in0=y[:, mf, :, :],
                scalar1=MASK, scalar2=None, op0=ALU.bitwise_and)
            nc.scalar.activation(
                s[:, m, :, :], ph[:, mf, :, :], AF.Sin,
                bias=negpi[:], scale=SCALE_SIN)
            if side == 1:
                amp(m)

    def amp(m):
        for j in range(2):
            nc.vector.tensor_scalar_mul(
                out=sqs[:, m, :, bass.ts(j, TQ)],
                in0=sq[:, m, :, bass.ts(j, TQ)],
                scalar1=cwv_sb[:, m, j:j + 1])

    side_pipeline(0, wq_sb, qsT_sb, qT, yq, phq, sq, TQ)
    # PE idles while the k-side DMA lands; keep HAM at K=8/8
    ps_gap = pp.tile([128, 128], F32, name="ps_gap", tag="ps")
    for _ in range(14):
        nc.tensor.matmul(ps_gap[:], junk[:], junk[:], start=True, stop=True)
    side_pipeline(1, wk_sb, ksT_sb, kT, yk, phk, sk, TK)

    # ---- scores (transposed layout: [k%128, kh, qi]), PSUM accumulate ----
    ps_a = [ps_sc.tile([128, TK], F32, name=f"ps_sc{a}", tag=f"ps_sc{a}")[:]
            for a in range(2)]
    for m in range(M):
        for kh in range(2):
            for j in range(2):
                for (qq, kq) in ((0, 1), (1, 0)):
                    nc.tensor.matmul(
                        ps_a[kh],
                        sk[:, m, kq, bass.ds(j * TK + kh * 128, 128)],
                        sqs[:, m, qq, bass.ts(j, TQ)],
                        start=(m == 0 and j == 0 and (qq, kq) == (0, 1)),
                        stop=(m == M - 1 and j == 1 and (qq, kq) == (1, 0)))
        if m < M - 1:
            ps_bridge = pp.tile([128, 128], F32, name="ps_bridge", tag="ps")
            for _ in range(10):
                nc.tensor.matmul(ps_bridge[:], junk[:], junk[:],
                                 start=True, stop=True)

    # ---- softmax (deferred normalization, on scoresT) ----
    attn_bf = sb.tile([128, 2, TQ], BF16)   # [k%128, khalf, qi] = exp(scoresT)
    rcp = sb.tile([128, 2], F32)
    for kh in range(2):
        nc.scalar.activation(attn_bf[:, kh, :], ps_a[kh], AF.Exp,
                             bias=0.0, scale=1.0)
    # row sums per qi-half: ones-column matmul over all k
    for a in range(2):
        sm = pj.tile([128, 1], F32, name=f"sm{a}", tag=f"pj{a}")
        for kh in range(2):
            nc.tensor.matmul(sm[:], attn_bf[:, kh, bass.ts(a, 128)],
                             junk[:, 0:1],
                             start=(kh == 0), stop=(kh == 1))
        nc.vector.reciprocal(rcp[:, a:a + 1], sm[:])

    # ---- out = attnT.T @ values, scaled by 1/rowsum ----
    # scale rides ACT Copy's free affine (ACT is idle by now); the store is
    # chunked so scale/DMA pipeline across the two HW queues
    for a in range(2):
        po = ps_out.tile([128, DV], F32)
        for kh in range(2):
            nc.tensor.matmul(po[:], attn_bf[:, kh, bass.ts(a, 128)],
                             vals_bf[:, kh, :],
                             start=(kh == 0), stop=(kh == 1))
        o = sb.tile([128, DV], F32, tag=f"o{a}")
        for c in range(2):
            nc.scalar.activation(o[:, bass.ts(c, 256)],
                                 po[:, bass.ts(c, 256)], AF.Copy,
                                 bias=0.0, scale=rcp[:, a:a + 1])
            eng = nc.sync if c == 0 else nc.scalar
            eng.dma_start(out[bass.ts(a, 128), bass.ts(c, 256)],
                          o[:, bass.ts(c, 256)])


def _shuffle(x):
    """(512, n) -> (128, 4, n) with [d%128, dchunk, i]."""
    return np.ascontiguousarray(x.reshape(4, 128, x.shape[1]).transpose(1, 0, 2))


def kernel(queries, keys, values, Wq, Wk, wv, _trace=False):
    if "g" not in _CACHE:
        _CACHE["g"] = _build_graph()
    nc = _CACHE["g"]

    cwv = (COEF[None, :, None] *
           wv.astype(np.float64).reshape(2, 128).T[:, None, :]).astype(np.float32)
    base = {
        "wq": _shuffle(Wq.astype(np.float16)),
        "wk": _shuffle(Wk.astype(np.float16)),
        "cwv": cwv,
    }
    in_maps = []
    for b in range(B):
        m = dict(base)
        m["qsT"] = _shuffle(queries[b].T.astype(np.float16))
        m["ksT"] = _shuffle(keys[b].T.astype(np.float16))
        v = values[b].astype(ml_dtypes.bfloat16)
        m["vals"] = np.ascontiguousarray(v.reshape(2, 128, DV).transpose(1, 0, 2))
        in_maps.append(m)
    kw = {"trace": True, "trace_cores": [0]} if _trace else {}
    res = run_bass_kernel_spmd(nc, in_maps, core_ids=list(range(B)), **kw)
    _CACHE["last"] = res
    return np.stack([res.results[b]["out"] for b in range(B)], axis=0)
